# revision 9
# baseline (speedup 1.0000x reference)
"""CrossAttention Trainium2 kernel (v6).

Sharding: 4 batch-groups x 2 head-groups on 8 cores. Core c handles
batches [2*(c//2), 2*(c//2)+1] and heads [6*(c%2) .. 6*(c%2)+6). The two
head-group partial outputs are summed (+bias) on the host.

Per-core math, pair-at-a-time (pair = 2 heads on the 128 partitions),
loop order: pair outer, batch inner — so the exp(attn_pos) tiles for a
pair are DMA'd once and reused by both batches (halves epos traffic).

Per (pair p, batch b, k-chunk, q-half) unit:
  s[128, 1024] = (S^T_h0 qh | S^T_h1 qh)  two concurrent K=64 matmuls
                 (row groups 0-1 / 2-3 via base_partition 0 / 64)
  pr = exp(s - ln256)          one ACT op   [128, 1024]
  pt = pr * ep                 one DVE op   [128, 1024] bf16 2x mode
  o_h[65, qh] += V_aug^T pt_h  psum-accumulated over k (row 64 = rowsum)

PSUM budget (8 banks): s-ring 2 x [128,1024] f32 (4 banks, also serves
QK-proj / V-proj / out-proj burst tiles), o0 + o1 [65,1024] f32
single-buffered (4 banks).

QK projections for the next (p,b) run as dense 6-matmul bursts through
the s-ring, spread at k-iteration ends; V projections and the b0 output
projection are spread the same way. Normalization (1/rowsum broadcast
via a DRAM bounce) for pair p runs during pair p+1.
"""

import numpy as np

B, L, DIM, H, HD = 8, 1024, 768, 12, 64
NCORES = 8
BL = 2            # batches per core
HC = 6            # heads per core
NPC = 3           # head-pairs per core
HDIM = 384        # head-group slice of DIM
CP = DIM // 128   # 6 contraction chunks
DP = HDIM // 128  # 3 chunks of the per-core head dim
KC = L // 128     # 8 k-chunks
SCALE = HD ** -0.5
LN_OFF = float(np.log(256.0))

_CACHE = {}


def _build():
    import concourse.bass as bass
    import concourse.mybir as mybir
    import concourse.tile as tile
    from concourse import bacc

    f32 = mybir.dt.float32
    f16 = mybir.dt.float16
    bf16 = mybir.dt.bfloat16
    AF = mybir.ActivationFunctionType

    nc = bacc.Bacc("TRN2", target_bir_lowering=False, debug=False)

    qT = nc.dram_tensor("qT", [BL, DIM, L], f16, kind="ExternalInput")
    kvT = nc.dram_tensor("kvT", [BL, DIM, L], f16, kind="ExternalInput")
    wq = nc.dram_tensor("wq", [DIM, HDIM], f16, kind="ExternalInput")   # [c, d]
    wk = nc.dram_tensor("wk", [DIM, HDIM], f16, kind="ExternalInput")   # [c, d]
    wv = nc.dram_tensor("wv", [DIM, HDIM], f16, kind="ExternalInput")   # [c, d]
    wp = nc.dram_tensor("wp", [HDIM, DIM], f16, kind="ExternalInput")   # [d, e]
    epd = nc.dram_tensor("ep", [NPC, KC, 2, 128, L], bf16, kind="ExternalInput")
    out = nc.dram_tensor("out", [BL, L, DIM], f16, kind="ExternalOutput")
    rscr = nc.dram_tensor("rs_scratch", [NPC * 4, L], f32)

    with tile.TileContext(nc) as tc:
        with tc.tile_pool(name="persist", bufs=1) as persist:
            q_sb = persist.tile([128, BL, CP, L], f16)
            kv_sb = persist.tile([128, BL, CP, L], f16)
            wq_sb = persist.tile([128, CP, HDIM], f16)
            wk_sb = persist.tile([128, CP, HDIM], f16)
            wv_sb = persist.tile([128, CP, HDIM], f16)
            wp_sb = persist.tile([128, DP, DIM], f16)
            XT = persist.tile([128, BL, DP, L], f16)
            Vt = [
                [
                    persist.tile([128, HC, HD + 1], f16, name=f"Vt{b}_{k}")
                    for k in range(KC)
                ]
                for b in range(BL)
            ]
            # engine ops need partition-0-based tiles: one rs/recip tile
            # per reciprocal batch (pair 0, pair 1, pair2-b0, pair2-b1)
            rs_t = [
                persist.tile([4, L], f32, name="rs0"),
                persist.tile([4, L], f32, name="rs1"),
                persist.tile([2, L], f32, name="rs2a"),
                persist.tile([2, L], f32, name="rs2b"),
            ]
            recip_t = [
                persist.tile([4, L], f32, name="rc0"),
                persist.tile([4, L], f32, name="rc1"),
                persist.tile([2, L], f32, name="rc2a"),
                persist.tile([2, L], f32, name="rc2b"),
            ]
            expb = persist.tile([128, 1], f32)
            nc.vector.memset(expb[:], -LN_OFF)
            warm_w = persist.tile([128, 128], f16)
            warm_x = persist.tile([128, 512], f16)
            nc.vector.memset(warm_w[:], 0.0)
            nc.vector.memset(warm_x[:], 0.0)

            with (
                tc.tile_pool(name="psS", bufs=2, space="PSUM") as psS,
                tc.tile_pool(name="psO", bufs=1, space="PSUM") as psO,
                tc.tile_pool(name="qtp", bufs=2) as qtp,
                tc.tile_pool(name="ktp", bufs=2) as ktp,
                tc.tile_pool(name="eposp", bufs=16) as eposp,
                tc.tile_pool(name="prp", bufs=3) as prp,
                tc.tile_pool(name="ptp", bufs=3) as ptp,
                tc.tile_pool(name="xtup", bufs=5) as xtup,
                tc.tile_pool(name="bcp", bufs=2) as bcp,
                tc.tile_pool(name="outp", bufs=2) as outp,
            ):
                ep_tiles = {}

                def ep_dma(p, kc, qh):
                    t = eposp.tile(
                        [128, L], bf16, tag="ep", name=f"ep{p}_{kc}_{qh}"
                    )
                    nc.sync.dma_start(t[:], epd[p, kc, qh])
                    ep_tiles[(p, kc, qh)] = t

                # ---- head DMAs (single HW ring, consumption order), with
                # ---- the first ep chunks interleaved so pair 0 can start
                nc.sync.dma_start(
                    kv_sb[:, 0], kvT[0].rearrange("(a p) q -> p a q", p=128)
                )
                nc.sync.dma_start(wv_sb[:], wv.rearrange("(a p) d -> p a d", p=128))
                ep_dma(0, 0, 0)
                ep_dma(0, 0, 1)
                nc.sync.dma_start(wq_sb[:], wq.rearrange("(a p) d -> p a d", p=128))
                nc.sync.dma_start(wk_sb[:], wk.rearrange("(a p) d -> p a d", p=128))
                ep_dma(0, 1, 0)
                ep_dma(0, 1, 1)
                nc.sync.dma_start(
                    q_sb[:, 0], qT[0].rearrange("(a p) q -> p a q", p=128)
                )
                ep_dma(0, 2, 0)
                ep_dma(0, 2, 1)
                nc.sync.dma_start(
                    kv_sb[:, 1], kvT[1].rearrange("(a p) q -> p a q", p=128)
                )
                ep_dma(0, 3, 0)
                ep_dma(0, 3, 1)
                nc.sync.dma_start(
                    q_sb[:, 1], qT[1].rearrange("(a p) q -> p a q", p=128)
                )
                for kc in range(4, KC):
                    ep_dma(0, kc, 0)
                    ep_dma(0, kc, 1)
                nc.sync.dma_start(wp_sb[:], wp.rearrange("(a p) d -> p a d", p=128))

                # ---- warmup burst: keep the PE HAM gate hot through the
                # ---- initial DMA wall
                wps = psS.tile([128, 512], f32, tag="s")
                for _ in range(24):
                    nc.tensor.matmul(wps[:], warm_w[:], warm_x[:])

                def v_proj(b, k):
                    ps = psS.tile([128, HDIM], f32, tag="s", name=f"v{b}{k}")
                    for c in range(CP):
                        nc.tensor.matmul(
                            ps[:],
                            kv_sb[:, b, c, k * 128:(k + 1) * 128],
                            wv_sb[:, c, :],
                            start=(c == 0),
                            stop=(c == CP - 1),
                        )
                    nc.vector.memset(Vt[b][k][:, :, HD:HD + 1], 1.0)
                    nc.vector.tensor_copy(
                        Vt[b][k][:, :, 0:HD],
                        ps.rearrange("p (h d) -> p h d", d=HD),
                    )

                def qk_burst(dst, w_sb, x_sb, b, p, hf):
                    ps = psS.tile([128, 512], f32, tag="s", name=f"qk{b}{p}{hf}")
                    for c in range(CP):
                        nc.tensor.matmul(
                            ps[:],
                            w_sb[:, c, p * 128:(p + 1) * 128],
                            x_sb[:, b, c, hf * 512:(hf + 1) * 512],
                            start=(c == 0),
                            stop=(c == CP - 1),
                        )
                    nc.vector.tensor_copy(dst[:, hf * 512:(hf + 1) * 512], ps[:])

                def out_proj(b, qc):
                    ps = psS.tile([128, DIM], f32, tag="s", name=f"op{b}{qc}")
                    for d in range(DP):
                        for lo, sz in ((0, 512), (512, 256)):
                            nc.tensor.matmul(
                                ps[:, lo:lo + sz],
                                XT[:, b, d, qc * 128:(qc + 1) * 128],
                                wp_sb[:, d, lo:lo + sz],
                                start=(d == 0),
                                stop=(d == DP - 1),
                            )
                    ot = outp.tile([128, DIM], f16, tag="ot", name=f"ot{b}{qc}")
                    nc.vector.tensor_copy(ot[:], ps[:])
                    nc.sync.dma_start(out[b, qc * 128:(qc + 1) * 128, :], ot[:])

                xtu_map = {}

                def normalize(p, b, sub):
                    row = p * 4 + b * 2 + sub
                    bc = bcp.tile([64, L], f32, tag="bc", name=f"bc{row}")
                    nc.sync.dma_start(bc[:], rscr[row:row + 1, :].broadcast_to([64, L]))
                    nc.vector.tensor_mul(
                        XT[sub * 64:(sub + 1) * 64, b, p, :],
                        xtu_map[(p, b, sub)][0:64, :],
                        bc[:],
                    )

                # ---- prologue: V projections + first QK while inputs stream
                qt_cur = qtp.tile([128, L], f16, tag="qt", name="qt00")
                kt_cur = ktp.tile([128, L], f16, tag="kt", name="kt00")
                for k in range(KC):
                    v_proj(0, k)
                for hf in range(2):
                    qk_burst(qt_cur, wq_sb, q_sb, 0, 0, hf)
                for hf in range(2):
                    qk_burst(kt_cur, wk_sb, kv_sb, 0, 0, hf)
                for k in range(4):
                    v_proj(1, k)

                # ---- main loop: pair outer, batch inner
                iters = [(p, b) for p in range(NPC) for b in range(BL)]
                qt_next = kt_next = None
                for it, (p, b) in enumerate(iters):
                    o_ps0 = psO.tile([HD + 1, L], f32, tag="o0", name=f"o0_{p}{b}")
                    o_ps1 = psO.tile([HD + 1, L], f32, tag="o1", name=f"o1_{p}{b}")

                    # prefetch next pair's ep tiles (slots free as this
                    # pair's b1 pass consumes the old ones)
                    if b == 1 and p + 1 < NPC:
                        for kc in range(KC):
                            for qh in range(2):
                                ep_dma(p + 1, kc, qh)

                    # reciprocals for the previous pair become available now
                    if b == 0 and p > 0:
                        r0 = (p - 1) * 4
                        nc.vector.reciprocal_approx_fast(
                            recip_t[p - 1][:], rs_t[p - 1][:]
                        )
                        nc.sync.dma_start(rscr[r0:r0 + 4, :], recip_t[p - 1][:])

                    # filler jobs for this iteration, drained at k-ends
                    jobs = []
                    if (p, b) == (0, 0):
                        for k in range(4, KC):
                            jobs.append(lambda k=k: v_proj(1, k))
                    if it + 1 < len(iters):
                        np_, nb = iters[it + 1]
                        qt_next = qtp.tile([128, L], f16, tag="qt", name=f"qt{np_}{nb}")
                        kt_next = ktp.tile([128, L], f16, tag="kt", name=f"kt{np_}{nb}")
                        for hf in range(2):
                            jobs.append(
                                lambda hf=hf, t=qt_next, nb=nb, np_=np_:
                                qk_burst(t, wq_sb, q_sb, nb, np_, hf)
                            )
                        for hf in range(2):
                            jobs.append(
                                lambda hf=hf, t=kt_next, nb=nb, np_=np_:
                                qk_burst(t, wk_sb, kv_sb, nb, np_, hf)
                            )
                    if b == 0 and p > 0:
                        for b_ in range(BL):
                            for sub in range(2):
                                jobs.append(
                                    lambda b_=b_, sub=sub: normalize(p - 1, b_, sub)
                                )
                    if p == NPC - 1 and b == 1:
                        # pair (2, b0) normalization must precede the b0
                        # output projection below (XT read-after-write)
                        nc.vector.reciprocal_approx_fast(recip_t[2][:], rs_t[2][:])
                        nc.sync.dma_start(rscr[8:10, :], recip_t[2][:])
                        jobs.append(lambda: normalize(NPC - 1, 0, 0))
                        jobs.append(lambda: normalize(NPC - 1, 0, 1))
                        for qc in range(KC):
                            jobs.append(lambda qc=qc: out_proj(0, qc))

                    nj = 0

                    def drain(n):
                        nonlocal nj
                        for _ in range(n):
                            if nj >= len(jobs):
                                return
                            jobs[nj]()
                            nj += 1

                    h0, h1 = 2 * p, 2 * p + 1
                    for k in range(KC):
                        kt_sl = slice(k * 128, (k + 1) * 128)
                        for qh in range(2):
                            qs = slice(qh * 512, (qh + 1) * 512)
                            s = psS.tile([128, L], f32, tag="s", name=f"s{p}{b}{k}{qh}")
                            nc.tensor.matmul(
                                s[:, 0:512], kt_cur[0:64, kt_sl], qt_cur[0:64, qs]
                            )
                            nc.tensor.matmul(
                                s[:, 512:L], kt_cur[64:128, kt_sl], qt_cur[64:128, qs]
                            )
                            pr = prp.tile([128, L], bf16, tag="pr")
                            nc.scalar.activation(pr[:], s[:], AF.Exp, bias=expb[:])
                            pt = ptp.tile([128, L], bf16, tag="pt")
                            nc.vector.tensor_mul(pt[:], pr[:], ep_tiles[(p, k, qh)][:])
                            nc.tensor.matmul(
                                o_ps0[:, qs], Vt[b][k][:, h0, :], pt[:, 0:512],
                                start=(k == 0), stop=(k == KC - 1),
                            )
                            nc.tensor.matmul(
                                o_ps1[:, qs], Vt[b][k][:, h1, :], pt[:, 512:L],
                                start=(k == 0), stop=(k == KC - 1),
                            )
                        drain(1 if len(jobs) <= KC else 2)
                    drain(len(jobs))

                    # evacuate o psum: copy to SBUF, export rowsum row
                    for sub, o_ps in ((0, o_ps0), (1, o_ps1)):
                        xtu = xtup.tile(
                            [HD + 1, L], f32, tag="xtu", name=f"xtu{p}{b}{sub}"
                        )
                        nc.vector.tensor_copy(xtu[:], o_ps[:])
                        if p < 2:
                            rt, rr = rs_t[p], b * 2 + sub
                        else:
                            rt, rr = rs_t[2 + b], sub
                        nc.sync.dma_start(rt[rr:rr + 1, :], xtu[HD:HD + 1, :])
                        xtu_map[(p, b, sub)] = xtu

                    qt_cur, kt_cur = qt_next, kt_next

                # ---- tail: last two heads' normalize + second batch out-proj
                nc.vector.reciprocal_approx_fast(recip_t[3][:], rs_t[3][:])
                nc.sync.dma_start(rscr[10:12, :], recip_t[3][:])
                normalize(NPC - 1, 1, 0)
                normalize(NPC - 1, 1, 1)
                for qc in range(KC):
                    out_proj(1, qc)

    nc.compile()
    return nc


def _get_nc():
    if "nc" not in _CACHE:
        _CACHE["nc"] = _build()
    return _CACHE["nc"]


def _host_prep(q, kv, attn_pos, Wq, Wkv, Wproj, bproj):
    import ml_dtypes

    q = np.asarray(q, dtype=np.float32)
    kv = np.asarray(kv, dtype=np.float32)
    attn_pos = np.asarray(attn_pos, dtype=np.float32)
    Wq = np.asarray(Wq, dtype=np.float32)
    Wkv = np.asarray(Wkv, dtype=np.float32)
    Wproj = np.asarray(Wproj, dtype=np.float32)

    wq16 = np.ascontiguousarray((Wq * SCALE).T).astype(np.float16)   # [c, d]
    wk16 = np.ascontiguousarray(Wkv[:DIM].T).astype(np.float16)      # [c, d]
    wv16 = np.ascontiguousarray(Wkv[DIM:].T).astype(np.float16)      # [c, d]
    wp16 = np.ascontiguousarray(Wproj.T).astype(np.float16)          # [d, e]
    E = np.exp(attn_pos[0]).transpose(0, 2, 1)                       # [h, k, q]

    qT = np.ascontiguousarray(q.transpose(0, 2, 1)).astype(np.float16)
    kvT = np.ascontiguousarray(kv.transpose(0, 2, 1)).astype(np.float16)

    ep_g = []
    for g in range(2):
        Eg = E[g * HC:(g + 1) * HC]
        ep_g.append(
            np.ascontiguousarray(
                Eg.reshape(NPC, 2, KC, 128, 2, 512)
                .transpose(0, 2, 4, 3, 1, 5)
                .reshape(NPC, KC, 2, 128, L)
            ).astype(ml_dtypes.bfloat16)
        )
    w_g = []
    for g in range(2):
        hs = slice(g * HDIM, (g + 1) * HDIM)
        w_g.append({
            "wq": np.ascontiguousarray(wq16[:, hs]),
            "wk": np.ascontiguousarray(wk16[:, hs]),
            "wv": np.ascontiguousarray(wv16[:, hs]),
            "wp": np.ascontiguousarray(wp16[hs, :]),
        })

    in_maps = []
    for c in range(NCORES):
        bg, g = c // 2, c % 2
        m = dict(w_g[g])
        m["ep"] = ep_g[g]
        m["qT"] = np.ascontiguousarray(qT[2 * bg:2 * bg + 2])
        m["kvT"] = np.ascontiguousarray(kvT[2 * bg:2 * bg + 2])
        in_maps.append(m)
    return in_maps


def kernel(q, kv, attn_pos, Wq, Wkv, Wproj, bproj):
    from concourse.bass_utils import run_bass_kernel_spmd

    bproj = np.asarray(bproj, dtype=np.float32)
    nc = _get_nc()
    in_maps = _host_prep(q, kv, attn_pos, Wq, Wkv, Wproj, bproj)
    res = run_bass_kernel_spmd(nc, in_maps, list(range(NCORES)))
    out = np.empty((B, L, DIM), np.float32)
    for b in range(B):
        bg = b // 2
        out[b] = (
            res.results[2 * bg]["out"][b % 2].astype(np.float32)
            + res.results[2 * bg + 1]["out"][b % 2].astype(np.float32)
            + bproj[None, :]
        )
    return out


# revision 20
# speedup vs baseline: 1.0401x; 1.0401x over previous
"""CrossAttention Trainium2 kernel (v6).

Sharding: 4 batch-groups x 2 head-groups on 8 cores. Core c handles
batches [2*(c//2), 2*(c//2)+1] and heads [6*(c%2) .. 6*(c%2)+6). The two
head-group partial outputs are summed (+bias) on the host.

Per-core math, pair-at-a-time (pair = 2 heads on the 128 partitions),
loop order: pair outer, batch inner — so the exp(attn_pos) tiles for a
pair are DMA'd once and reused by both batches (halves epos traffic).

Per (pair p, batch b, k-chunk, q-half) unit:
  s[128, 1024] = (S^T_h0 qh | S^T_h1 qh)  two concurrent K=64 matmuls
                 (row groups 0-1 / 2-3 via base_partition 0 / 64)
  pr = exp(s - ln256)          one ACT op   [128, 1024]
  pt = pr * ep                 one DVE op   [128, 1024] bf16 2x mode
  o_h[65, qh] += V_aug^T pt_h  psum-accumulated over k (row 64 = rowsum)

PSUM budget (8 banks): s-ring 2 x [128,1024] f32 (4 banks, also serves
QK-proj / V-proj / out-proj burst tiles), o0 + o1 [65,1024] f32
single-buffered (4 banks).

QK projections for the next (p,b) run as dense 6-matmul bursts through
the s-ring, spread at k-iteration ends; V projections and the b0 output
projection are spread the same way. Normalization (1/rowsum broadcast
via a DRAM bounce) for pair p runs during pair p+1.
"""

import numpy as np

B, L, DIM, H, HD = 8, 1024, 768, 12, 64
NCORES = 8
BL = 2            # batches per core
HC = 6            # heads per core
NPC = 3           # head-pairs per core
HDIM = 384        # head-group slice of DIM
CP = DIM // 128   # 6 contraction chunks
DP = HDIM // 128  # 3 chunks of the per-core head dim
KC = L // 128     # 8 k-chunks
SCALE = HD ** -0.5
LN_OFF = float(np.log(256.0))

_CACHE = {}


def _build():
    import concourse.bass as bass
    import concourse.mybir as mybir
    import concourse.tile as tile
    from concourse import bacc

    f32 = mybir.dt.float32
    f16 = mybir.dt.float16
    bf16 = mybir.dt.bfloat16
    AF = mybir.ActivationFunctionType

    nc = bacc.Bacc("TRN2", target_bir_lowering=False, debug=False)

    qT = nc.dram_tensor("qT", [BL, DIM, L], f16, kind="ExternalInput")
    kvT = nc.dram_tensor("kvT", [BL, DIM, L], f16, kind="ExternalInput")
    wq = nc.dram_tensor("wq", [DIM, HDIM], f16, kind="ExternalInput")   # [c, d]
    wk = nc.dram_tensor("wk", [DIM, HDIM], f16, kind="ExternalInput")   # [c, d]
    wv = nc.dram_tensor("wv", [DIM, HDIM], f16, kind="ExternalInput")   # [c, d]
    wp = nc.dram_tensor("wp", [HDIM, DIM], f16, kind="ExternalInput")   # [d, e]
    epd = nc.dram_tensor("ep", [NPC, KC, 2, 128, L], bf16, kind="ExternalInput")
    out = nc.dram_tensor("out", [BL, L, DIM], f16, kind="ExternalOutput")
    rscr = nc.dram_tensor("rs_scratch", [NPC * 4, L], f32)

    with tile.TileContext(nc) as tc:
        with tc.tile_pool(name="persist", bufs=1) as persist:
            q_sb = persist.tile([128, BL, CP, L], f16)
            kv_sb = persist.tile([128, BL, CP, L], f16)
            wq_sb = persist.tile([128, CP, HDIM], f16)
            wk_sb = persist.tile([128, CP, HDIM], f16)
            wv_sb = persist.tile([128, CP, HDIM], f16)
            wp_sb = persist.tile([128, DP, DIM], f16)
            XT = persist.tile([128, BL, DP, L], f16)
            Vt = [
                [
                    persist.tile([128, HC, HD + 1], f16, name=f"Vt{b}_{k}")
                    for k in range(KC)
                ]
                for b in range(BL)
            ]
            # engine ops need partition-0-based tiles: one rs/recip tile
            # per reciprocal batch (pair 0, pair 1, pair2-b0, pair2-b1)
            rs_t = [
                persist.tile([4, L], f32, name="rs0"),
                persist.tile([4, L], f32, name="rs1"),
                persist.tile([2, L], f32, name="rs2a"),
            ]
            recip_t = [
                persist.tile([4, L], f32, name="rc0"),
                persist.tile([4, L], f32, name="rc1"),
                persist.tile([2, L], f32, name="rc2a"),
            ]
            # tail (2,b1) normalize is DMA-free: per-head reciprocal rows
            # feed a rank-1 fp32 matmul broadcast instead of a DRAM bounce
            rc32 = [
                persist.tile([1, L], f32, name="tl_rc0"),
                persist.tile([1, L], f32, name="tl_rc1"),
            ]
            ones32 = persist.tile([1, HD], f32)
            nc.vector.memset(ones32[:], 1.0)
            expb = persist.tile([128, 1], f32)
            nc.vector.memset(expb[:], -LN_OFF)
            warm_w = persist.tile([128, 128], f16)
            warm_x = persist.tile([128, 512], f16)
            nc.vector.memset(warm_w[:], 0.0)
            nc.vector.memset(warm_x[:], 0.0)

            with (
                tc.tile_pool(name="psS", bufs=2, space="PSUM") as psS,
                tc.tile_pool(name="psO", bufs=1, space="PSUM") as psO,
                tc.tile_pool(name="qtp", bufs=2) as qtp,
                tc.tile_pool(name="ktp", bufs=2) as ktp,
                tc.tile_pool(name="eposp", bufs=16) as eposp,
                tc.tile_pool(name="prp", bufs=3) as prp,
                tc.tile_pool(name="ptp", bufs=3) as ptp,
                tc.tile_pool(name="xtup", bufs=5) as xtup,
                tc.tile_pool(name="bcp", bufs=2) as bcp,
                tc.tile_pool(name="outp", bufs=2) as outp,
            ):
                ep_tiles = {}

                def ep_dma(p, kc, qh):
                    t = eposp.tile(
                        [128, L], bf16, tag="ep", name=f"ep{p}_{kc}_{qh}"
                    )
                    nc.sync.dma_start(t[:], epd[p, kc, qh])
                    ep_tiles[(p, kc, qh)] = t

                # ---- head DMAs (single HW ring, consumption order), with
                # ---- the first ep chunks interleaved so pair 0 can start
                nc.sync.dma_start(
                    kv_sb[:, 0], kvT[0].rearrange("(a p) q -> p a q", p=128)
                )
                nc.sync.dma_start(wv_sb[:], wv.rearrange("(a p) d -> p a d", p=128))
                ep_dma(0, 0, 0)
                ep_dma(0, 0, 1)
                nc.sync.dma_start(wq_sb[:], wq.rearrange("(a p) d -> p a d", p=128))
                nc.sync.dma_start(wk_sb[:], wk.rearrange("(a p) d -> p a d", p=128))
                ep_dma(0, 1, 0)
                ep_dma(0, 1, 1)
                nc.sync.dma_start(
                    q_sb[:, 0], qT[0].rearrange("(a p) q -> p a q", p=128)
                )
                ep_dma(0, 2, 0)
                ep_dma(0, 2, 1)
                nc.sync.dma_start(
                    kv_sb[:, 1], kvT[1].rearrange("(a p) q -> p a q", p=128)
                )
                ep_dma(0, 3, 0)
                ep_dma(0, 3, 1)
                nc.sync.dma_start(
                    q_sb[:, 1], qT[1].rearrange("(a p) q -> p a q", p=128)
                )
                for kc in range(4, KC):
                    ep_dma(0, kc, 0)
                    ep_dma(0, kc, 1)
                nc.sync.dma_start(wp_sb[:], wp.rearrange("(a p) d -> p a d", p=128))

                # ---- warmup burst: keep the PE HAM gate hot through the
                # ---- initial DMA wall
                wps = psS.tile([128, 512], f32, tag="s")
                for _ in range(24):
                    nc.tensor.matmul(wps[:], warm_w[:], warm_x[:])

                def v_proj(b, k):
                    ps = psS.tile([128, HDIM], f32, tag="s", name=f"v{b}{k}")
                    for c in range(CP):
                        nc.tensor.matmul(
                            ps[:],
                            kv_sb[:, b, c, k * 128:(k + 1) * 128],
                            wv_sb[:, c, :],
                            start=(c == 0),
                            stop=(c == CP - 1),
                        )
                    nc.vector.memset(Vt[b][k][:, :, HD:HD + 1], 1.0)
                    nc.vector.tensor_copy(
                        Vt[b][k][:, :, 0:HD],
                        ps.rearrange("p (h d) -> p h d", d=HD),
                    )

                def qk_burst(dst, w_sb, x_sb, b, p, hf):
                    ps = psS.tile([128, 512], f32, tag="s", name=f"qk{b}{p}{hf}")
                    for c in range(CP):
                        nc.tensor.matmul(
                            ps[:],
                            w_sb[:, c, p * 128:(p + 1) * 128],
                            x_sb[:, b, c, hf * 512:(hf + 1) * 512],
                            start=(c == 0),
                            stop=(c == CP - 1),
                        )
                    nc.vector.tensor_copy(dst[:, hf * 512:(hf + 1) * 512], ps[:])

                def out_proj(b, qc):
                    ps = psS.tile([128, DIM], f32, tag="s", name=f"op{b}{qc}")
                    for d in range(DP):
                        for lo, sz in ((0, 512), (512, 256)):
                            nc.tensor.matmul(
                                ps[:, lo:lo + sz],
                                XT[:, b, d, qc * 128:(qc + 1) * 128],
                                wp_sb[:, d, lo:lo + sz],
                                start=(d == 0),
                                stop=(d == DP - 1),
                            )
                    ot = outp.tile([128, DIM], f16, tag="ot", name=f"ot{b}{qc}")
                    nc.vector.tensor_copy(ot[:], ps[:])
                    nc.sync.dma_start(out[b, qc * 128:(qc + 1) * 128, :], ot[:])

                xtu_map = {}

                def normalize(p, b, sub):
                    row = p * 4 + b * 2 + sub
                    bc = bcp.tile([64, L], f32, tag="bc", name=f"bc{row}")
                    nc.gpsimd.dma_start(bc[:], rscr[row:row + 1, :].broadcast_to([64, L]))
                    nc.vector.tensor_mul(
                        XT[sub * 64:(sub + 1) * 64, b, p, :],
                        xtu_map[(p, b, sub)][0:64, :],
                        bc[:],
                    )

                # ---- prologue: V projections + first QK while inputs stream
                qt_cur = qtp.tile([128, L], f16, tag="qt", name="qt00")
                kt_cur = ktp.tile([128, L], f16, tag="kt", name="kt00")
                for k in range(KC):
                    v_proj(0, k)
                for hf in range(2):
                    qk_burst(qt_cur, wq_sb, q_sb, 0, 0, hf)
                for hf in range(2):
                    qk_burst(kt_cur, wk_sb, kv_sb, 0, 0, hf)
                for k in range(4):
                    v_proj(1, k)

                # ---- main loop: pair outer, batch inner
                iters = [(p, b) for p in range(NPC) for b in range(BL)]
                qt_next = kt_next = None
                for it, (p, b) in enumerate(iters):
                    o_ps0 = psO.tile([HD + 1, L], f32, tag="o0", name=f"o0_{p}{b}")
                    o_ps1 = psO.tile([HD + 1, L], f32, tag="o1", name=f"o1_{p}{b}")

                    # prefetch next pair's ep tiles (slots free as this
                    # pair's b1 pass consumes the old ones)
                    if b == 1 and p + 1 < NPC:
                        for kc in range(KC):
                            for qh in range(2):
                                ep_dma(p + 1, kc, qh)

                    # reciprocals for the previous pair become available now
                    if b == 0 and p > 0:
                        r0 = (p - 1) * 4
                        nc.vector.reciprocal_approx_fast(
                            recip_t[p - 1][:], rs_t[p - 1][:]
                        )
                        nc.gpsimd.dma_start(rscr[r0:r0 + 4, :], recip_t[p - 1][:])

                    # filler jobs for this iteration, drained at k-ends
                    jobs = []
                    if (p, b) == (0, 0):
                        for k in range(4, KC):
                            jobs.append(lambda k=k: v_proj(1, k))
                    if it + 1 < len(iters):
                        np_, nb = iters[it + 1]
                        qt_next = qtp.tile([128, L], f16, tag="qt", name=f"qt{np_}{nb}")
                        kt_next = ktp.tile([128, L], f16, tag="kt", name=f"kt{np_}{nb}")
                        for hf in range(2):
                            jobs.append(
                                lambda hf=hf, t=qt_next, nb=nb, np_=np_:
                                qk_burst(t, wq_sb, q_sb, nb, np_, hf)
                            )
                        for hf in range(2):
                            jobs.append(
                                lambda hf=hf, t=kt_next, nb=nb, np_=np_:
                                qk_burst(t, wk_sb, kv_sb, nb, np_, hf)
                            )
                    if b == 0 and p > 0:
                        for b_ in range(BL):
                            for sub in range(2):
                                jobs.append(
                                    lambda b_=b_, sub=sub: normalize(p - 1, b_, sub)
                                )
                    if p == NPC - 1 and b == 1:
                        # pair (2, b0) normalization must precede the b0
                        # output projection below (XT read-after-write)
                        nc.vector.reciprocal_approx_fast(recip_t[2][:], rs_t[2][:])
                        nc.gpsimd.dma_start(rscr[8:10, :], recip_t[2][:])
                        jobs.append(lambda: normalize(NPC - 1, 0, 0))
                        jobs.append(lambda: normalize(NPC - 1, 0, 1))
                        for qc in range(KC):
                            jobs.append(lambda qc=qc: out_proj(0, qc))

                    nj = 0

                    def drain(n):
                        nonlocal nj
                        for _ in range(n):
                            if nj >= len(jobs):
                                return
                            jobs[nj]()
                            nj += 1

                    h0, h1 = 2 * p, 2 * p + 1

                    # PV runs 2 units behind scores so the in-order PE queue
                    # never stalls on the exp->mul chain
                    pv_pend = []

                    def pv_issue(pk, pqh, ppt):
                        pqs = slice(pqh * 512, (pqh + 1) * 512)
                        nc.tensor.matmul(
                            o_ps0[:, pqs], Vt[b][pk][:, h0, :], ppt[:, 0:512],
                            start=(pk == 0), stop=(pk == KC - 1),
                        )
                        nc.tensor.matmul(
                            o_ps1[:, pqs], Vt[b][pk][:, h1, :], ppt[:, 512:L],
                            start=(pk == 0), stop=(pk == KC - 1),
                        )

                    for k in range(KC):
                        kt_sl = slice(k * 128, (k + 1) * 128)
                        for qh in range(2):
                            qs = slice(qh * 512, (qh + 1) * 512)
                            s = psS.tile([128, L], f32, tag="s", name=f"s{p}{b}{k}{qh}")
                            nc.tensor.matmul(
                                s[:, 0:512], kt_cur[0:64, kt_sl], qt_cur[0:64, qs]
                            )
                            nc.tensor.matmul(
                                s[:, 512:L], kt_cur[64:128, kt_sl], qt_cur[64:128, qs]
                            )
                            pr = prp.tile([128, L], bf16, tag="pr")
                            nc.scalar.activation(pr[:], s[:], AF.Exp, bias=expb[:])
                            pt = ptp.tile([128, L], bf16, tag="pt")
                            nc.vector.tensor_mul(pt[:], pr[:], ep_tiles[(p, k, qh)][:])
                            pv_pend.append((k, qh, pt))
                            if len(pv_pend) > 2:
                                pv_issue(*pv_pend.pop(0))
                        drain(1 if len(jobs) <= KC else 2)
                    for args in pv_pend:
                        pv_issue(*args)
                    drain(len(jobs))

                    # evacuate o psum: copy to SBUF, export rowsum row
                    last_it = it == len(iters) - 1
                    for sub, o_ps in ((0, o_ps0), (1, o_ps1)):
                        xtu = xtup.tile(
                            [HD + 1, L], f32, tag="xtu", name=f"xtu{p}{b}{sub}"
                        )
                        if last_it:
                            # ACT is idle here; its rowsums never leave SBUF
                            nc.scalar.copy(xtu[:], o_ps[:])
                        else:
                            nc.vector.tensor_copy(xtu[:], o_ps[:])
                            rt, rr = (rs_t[p], b * 2 + sub) if p < 2 else (rs_t[2], sub)
                            nc.gpsimd.dma_start(rt[rr:rr + 1, :], xtu[HD:HD + 1, :])
                        xtu_map[(p, b, sub)] = xtu

                    qt_cur, kt_cur = qt_next, kt_next

                # ---- tail: last two heads' normalize (no DRAM bounce), b1 out-proj
                for sub in range(2):
                    xtu = xtu_map[(NPC - 1, 1, sub)]
                    # reciprocal_approx_fast needs a partition-0 input: bounce
                    # the rowsum row through a free rs tile's row 0
                    rrow = rs_t[2] if sub == 0 else rs_t[0]
                    nc.gpsimd.dma_start(rrow[0:1, :], xtu[HD:HD + 1, :])
                    nc.vector.reciprocal_approx_fast(rc32[sub][:], rrow[0:1, :])
                    bc_ps = psS.tile([64, L], f32, tag="s", name=f"tbc{sub}")
                    for hf in range(2):
                        hs = slice(hf * 512, (hf + 1) * 512)
                        nc.tensor.matmul(bc_ps[:, hs], ones32[:], rc32[sub][0:1, hs])
                    nc.vector.tensor_mul(
                        XT[sub * 64:(sub + 1) * 64, 1, NPC - 1, :],
                        xtu[0:HD, :],
                        bc_ps[:],
                    )
                for qc in range(KC):
                    out_proj(1, qc)

    nc.compile()
    return nc


def _get_nc():
    if "nc" not in _CACHE:
        _CACHE["nc"] = _build()
    return _CACHE["nc"]


def _host_prep(q, kv, attn_pos, Wq, Wkv, Wproj, bproj):
    import ml_dtypes

    q = np.asarray(q, dtype=np.float32)
    kv = np.asarray(kv, dtype=np.float32)
    attn_pos = np.asarray(attn_pos, dtype=np.float32)
    Wq = np.asarray(Wq, dtype=np.float32)
    Wkv = np.asarray(Wkv, dtype=np.float32)
    Wproj = np.asarray(Wproj, dtype=np.float32)

    wq16 = np.ascontiguousarray((Wq * SCALE).T).astype(np.float16)   # [c, d]
    wk16 = np.ascontiguousarray(Wkv[:DIM].T).astype(np.float16)      # [c, d]
    wv16 = np.ascontiguousarray(Wkv[DIM:].T).astype(np.float16)      # [c, d]
    wp16 = np.ascontiguousarray(Wproj.T).astype(np.float16)          # [d, e]
    E = np.exp(attn_pos[0]).transpose(0, 2, 1)                       # [h, k, q]

    qT = np.ascontiguousarray(q.transpose(0, 2, 1)).astype(np.float16)
    kvT = np.ascontiguousarray(kv.transpose(0, 2, 1)).astype(np.float16)

    ep_g = []
    for g in range(2):
        Eg = E[g * HC:(g + 1) * HC]
        ep_g.append(
            np.ascontiguousarray(
                Eg.reshape(NPC, 2, KC, 128, 2, 512)
                .transpose(0, 2, 4, 3, 1, 5)
                .reshape(NPC, KC, 2, 128, L)
            ).astype(ml_dtypes.bfloat16)
        )
    w_g = []
    for g in range(2):
        hs = slice(g * HDIM, (g + 1) * HDIM)
        w_g.append({
            "wq": np.ascontiguousarray(wq16[:, hs]),
            "wk": np.ascontiguousarray(wk16[:, hs]),
            "wv": np.ascontiguousarray(wv16[:, hs]),
            "wp": np.ascontiguousarray(wp16[hs, :]),
        })

    in_maps = []
    for c in range(NCORES):
        bg, g = c // 2, c % 2
        m = dict(w_g[g])
        m["ep"] = ep_g[g]
        m["qT"] = np.ascontiguousarray(qT[2 * bg:2 * bg + 2])
        m["kvT"] = np.ascontiguousarray(kvT[2 * bg:2 * bg + 2])
        in_maps.append(m)
    return in_maps


def kernel(q, kv, attn_pos, Wq, Wkv, Wproj, bproj):
    from concourse.bass_utils import run_bass_kernel_spmd

    bproj = np.asarray(bproj, dtype=np.float32)
    nc = _get_nc()
    in_maps = _host_prep(q, kv, attn_pos, Wq, Wkv, Wproj, bproj)
    res = run_bass_kernel_spmd(nc, in_maps, list(range(NCORES)))
    out = np.empty((B, L, DIM), np.float32)
    for b in range(B):
        bg = b // 2
        out[b] = (
            res.results[2 * bg]["out"][b % 2].astype(np.float32)
            + res.results[2 * bg + 1]["out"][b % 2].astype(np.float32)
            + bproj[None, :]
        )
    return out


# revision 34
# speedup vs baseline: 1.1757x; 1.1304x over previous
"""CrossAttention Trainium2 kernel (v6).

Sharding: 4 batch-groups x 2 head-groups on 8 cores. Core c handles
batches [2*(c//2), 2*(c//2)+1] and heads [6*(c%2) .. 6*(c%2)+6). The two
head-group partial outputs are summed (+bias) on the host.

Per-core math, pair-at-a-time (pair = 2 heads on the 128 partitions),
loop order: pair outer, batch inner — so the exp(attn_pos) tiles for a
pair are DMA'd once and reused by both batches (halves epos traffic).

Per (pair p, batch b, k-chunk, q-half) unit:
  s[128, 1024] = (S^T_h0 qh | S^T_h1 qh)  two concurrent K=64 matmuls
                 (row groups 0-1 / 2-3 via base_partition 0 / 64)
  pr = exp(s - ln256)          one ACT op   [128, 1024]
  pt = pr * ep                 one DVE op   [128, 1024] bf16 2x mode
  o_h[65, qh] += V_aug^T pt_h  psum-accumulated over k (row 64 = rowsum)

PSUM budget (8 banks): s-ring 2 x [128,1024] f32 (4 banks, also serves
QK-proj / V-proj / out-proj burst tiles), o0 + o1 [65,1024] f32
single-buffered (4 banks).

QK projections for the next (p,b) run as dense 6-matmul bursts through
the s-ring, spread at k-iteration ends; V projections and the b0 output
projection are spread the same way. Normalization (1/rowsum broadcast
via a DRAM bounce) for pair p runs during pair p+1.
"""

import numpy as np

B, L, DIM, H, HD = 8, 1024, 768, 12, 64
NCORES = 8
BL = 2            # batches per core
HC = 6            # heads per core
NPC = 3           # head-pairs per core
HDIM = 384        # head-group slice of DIM
CP = DIM // 128   # 6 contraction chunks
DP = HDIM // 128  # 3 chunks of the per-core head dim
KC = L // 128     # 8 k-chunks
SCALE = HD ** -0.5
LN_OFF = float(np.log(256.0))

_CACHE = {}


def _build():
    import concourse.bass as bass
    import concourse.mybir as mybir
    import concourse.tile as tile
    from concourse import bacc

    f32 = mybir.dt.float32
    f16 = mybir.dt.float16
    bf16 = mybir.dt.bfloat16
    AF = mybir.ActivationFunctionType

    nc = bacc.Bacc("TRN2", target_bir_lowering=False, debug=False)

    qT = nc.dram_tensor("qT", [BL, DIM, L], f16, kind="ExternalInput")
    kvT = nc.dram_tensor("kvT", [BL, DIM, L], f16, kind="ExternalInput")
    wq = nc.dram_tensor("wq", [DIM, HDIM], f16, kind="ExternalInput")   # [c, d]
    wk = nc.dram_tensor("wk", [DIM, HDIM], f16, kind="ExternalInput")   # [c, d]
    wv = nc.dram_tensor("wv", [DIM, HDIM], f16, kind="ExternalInput")   # [c, d]
    wp = nc.dram_tensor("wp", [HDIM, DIM], f16, kind="ExternalInput")   # [d, e]
    epd = nc.dram_tensor("ep", [NPC, KC, 2, 128, L], bf16, kind="ExternalInput")
    out = nc.dram_tensor("out", [BL, L, DIM], f16, kind="ExternalOutput")
    rscr = nc.dram_tensor("rs_scratch", [NPC * 4, L], f32)

    with tile.TileContext(nc) as tc:
        with tc.tile_pool(name="persist", bufs=1) as persist:
            q_sb = persist.tile([128, BL, CP, L], f16)
            kv_sb = persist.tile([128, BL, CP, L], f16)
            wq_sb = persist.tile([128, CP, HDIM], f16)
            wk_sb = persist.tile([128, CP, HDIM], f16)
            wv_sb = persist.tile([128, CP, HDIM], f16)
            wp_sb = persist.tile([128, DP, DIM], f16)
            XT = persist.tile([128, BL, DP, L], f16)
            Vt = [
                [
                    persist.tile([128, HC, HD + 1], f16, name=f"Vt{b}_{k}")
                    for k in range(KC)
                ]
                for b in range(BL)
            ]
            # engine ops need partition-0-based tiles: one rs/recip tile
            # per reciprocal batch (pair 0, pair 1, pair2-b0, pair2-b1)
            rs_t = [
                persist.tile([4, L], f32, name="rs0"),
                persist.tile([4, L], f32, name="rs1"),
                persist.tile([2, L], f32, name="rs2a"),
            ]
            recip_t = [
                persist.tile([4, L], f32, name="rc0"),
                persist.tile([4, L], f32, name="rc1"),
                persist.tile([2, L], f32, name="rc2a"),
            ]
            # pair-2 normalize skips the DRAM broadcast: bounce the rowsum
            # row to partition 0, reciprocal, cast to bf16 via a casting
            # gpsimd DMA, then a rank-1 bf16 matmul broadcasts it over the
            # 64 head dims
            rc32 = [
                persist.tile([1, L], f32, name="tl_rc0"),
                persist.tile([1, L], f32, name="tl_rc1"),
            ]
            rc16 = [persist.tile([1, L], bf16, name=f"rc16_{i}") for i in range(4)]
            ones16 = persist.tile([1, HD], bf16)
            nc.vector.memset(ones16[:], 1.0)
            expb = persist.tile([128, 1], f32)
            nc.vector.memset(expb[:], -LN_OFF)
            warm_w = persist.tile([128, 128], f16)
            warm_x = persist.tile([128, 512], f16)
            nc.vector.memset(warm_w[:], 0.0)
            nc.vector.memset(warm_x[:], 0.0)

            with (
                tc.tile_pool(name="psS", bufs=2, space="PSUM") as psS,
                tc.tile_pool(name="psO", bufs=1, space="PSUM") as psO,
                tc.tile_pool(name="qtp", bufs=2) as qtp,
                tc.tile_pool(name="ktp", bufs=2) as ktp,
                tc.tile_pool(name="eposp", bufs=16) as eposp,
                tc.tile_pool(name="prp", bufs=3) as prp,
                tc.tile_pool(name="ptp", bufs=3) as ptp,
                tc.tile_pool(name="xtup", bufs=4) as xtup,
                tc.tile_pool(name="bcp", bufs=1) as bcp,
                tc.tile_pool(name="outp", bufs=2) as outp,
            ):
                ep_tiles = {}

                def ep_dma(p, kc, qh):
                    t = eposp.tile(
                        [128, L], bf16, tag="ep", name=f"ep{p}_{kc}_{qh}"
                    )
                    nc.sync.dma_start(t[:], epd[p, kc, qh])
                    ep_tiles[(p, kc, qh)] = t

                # ---- head DMAs (single HW ring, consumption order), with
                # ---- the first ep chunks interleaved so pair 0 can start
                nc.sync.dma_start(
                    kv_sb[:, 0], kvT[0].rearrange("(a p) q -> p a q", p=128)
                )
                nc.sync.dma_start(wv_sb[:], wv.rearrange("(a p) d -> p a d", p=128))
                ep_dma(0, 0, 0)
                ep_dma(0, 0, 1)
                nc.sync.dma_start(wq_sb[:], wq.rearrange("(a p) d -> p a d", p=128))
                nc.sync.dma_start(wk_sb[:], wk.rearrange("(a p) d -> p a d", p=128))
                ep_dma(0, 1, 0)
                ep_dma(0, 1, 1)
                nc.sync.dma_start(
                    q_sb[:, 0], qT[0].rearrange("(a p) q -> p a q", p=128)
                )
                ep_dma(0, 2, 0)
                ep_dma(0, 2, 1)
                nc.sync.dma_start(
                    kv_sb[:, 1], kvT[1].rearrange("(a p) q -> p a q", p=128)
                )
                ep_dma(0, 3, 0)
                ep_dma(0, 3, 1)
                nc.sync.dma_start(
                    q_sb[:, 1], qT[1].rearrange("(a p) q -> p a q", p=128)
                )
                for kc in range(4, KC):
                    ep_dma(0, kc, 0)
                    ep_dma(0, kc, 1)
                nc.sync.dma_start(wp_sb[:], wp.rearrange("(a p) d -> p a d", p=128))

                # ---- warmup burst: keep the PE HAM gate hot through the
                # ---- initial DMA wall
                wps = psS.tile([128, 512], f32, tag="s")
                for _ in range(24):
                    nc.tensor.matmul(wps[:], warm_w[:], warm_x[:])

                def v_proj(b, k):
                    ps = psS.tile([128, HDIM], f32, tag="s", name=f"v{b}{k}")
                    for c in range(CP):
                        nc.tensor.matmul(
                            ps[:],
                            kv_sb[:, b, c, k * 128:(k + 1) * 128],
                            wv_sb[:, c, :],
                            start=(c == 0),
                            stop=(c == CP - 1),
                        )
                    nc.vector.memset(Vt[b][k][:, :, HD:HD + 1], 1.0)
                    nc.vector.tensor_copy(
                        Vt[b][k][:, :, 0:HD],
                        ps.rearrange("p (h d) -> p h d", d=HD),
                    )

                def qk_burst(dst, w_sb, x_sb, b, p, hf):
                    ps = psS.tile([128, 512], f32, tag="s", name=f"qk{b}{p}{hf}")
                    for c in range(CP):
                        nc.tensor.matmul(
                            ps[:],
                            w_sb[:, c, p * 128:(p + 1) * 128],
                            x_sb[:, b, c, hf * 512:(hf + 1) * 512],
                            start=(c == 0),
                            stop=(c == CP - 1),
                        )
                    nc.vector.tensor_copy(dst[:, hf * 512:(hf + 1) * 512], ps[:])

                def out_proj(b, qc):
                    ps = psS.tile([128, DIM], f32, tag="s", name=f"op{b}{qc}")
                    for d in range(DP):
                        for lo, sz in ((0, 512), (512, 256)):
                            nc.tensor.matmul(
                                ps[:, lo:lo + sz],
                                XT[:, b, d, qc * 128:(qc + 1) * 128],
                                wp_sb[:, d, lo:lo + sz],
                                start=(d == 0),
                                stop=(d == DP - 1),
                            )
                    ot = outp.tile([128, DIM], f16, tag="ot", name=f"ot{b}{qc}")
                    nc.vector.tensor_copy(ot[:], ps[:])
                    nc.sync.dma_start(out[b, qc * 128:(qc + 1) * 128, :], ot[:])

                xtu_map = {}

                def normalize2_ph1(b, sub):
                    # pair-2 normalize, latency phase: rowsum row to
                    # partition 0, reciprocal, bf16 cast via casting DMA
                    xtu = xtu_map[(NPC - 1, b, sub)]
                    i = b * 2 + sub
                    bounce = [rs_t[2], recip_t[2], rs_t[0], rs_t[1]][i]
                    rcp = [rc32[0], rc32[1], recip_t[0], recip_t[1]][i]
                    nc.gpsimd.dma_start(bounce[0:1, :], xtu[HD:HD + 1, :])
                    nc.vector.reciprocal_approx_fast(rcp[0:1, :], bounce[0:1, :])
                    nc.gpsimd.dma_start(rc16[i][:], rcp[0:1, :])  # f32 -> bf16

                def normalize2_ph2(b, sub):
                    # broadcast 1/rowsum over 64 partitions + normalize;
                    # issued well after ph1 so the PE queue never waits
                    xtu = xtu_map[(NPC - 1, b, sub)]
                    i = b * 2 + sub
                    bc_ps = psS.tile([HD, L], f32, tag="s", name=f"nbc{i}")
                    for hf in range(2):
                        hs = slice(hf * 512, (hf + 1) * 512)
                        nc.tensor.matmul(bc_ps[:, hs], ones16[:], rc16[i][0:1, hs])
                    nc.vector.tensor_mul(
                        XT[sub * 64:(sub + 1) * 64, b, NPC - 1, :],
                        xtu[0:HD, :],
                        bc_ps[:],
                    )

                def normalize(p, b, sub):
                    row = p * 4 + b * 2 + sub
                    bc = bcp.tile([64, L], f32, tag="bc", name=f"bc{row}")
                    nc.gpsimd.dma_start(bc[:], rscr[row:row + 1, :].broadcast_to([64, L]))
                    nc.vector.tensor_mul(
                        XT[sub * 64:(sub + 1) * 64, b, p, :],
                        xtu_map[(p, b, sub)][0:64, :],
                        bc[:],
                    )

                # ---- prologue: V projections + first QK while inputs stream
                qt_cur = qtp.tile([128, L], f16, tag="qt", name="qt00")
                kt_cur = ktp.tile([128, L], f16, tag="kt", name="kt00")
                for k in range(KC):
                    v_proj(0, k)
                for hf in range(2):
                    qk_burst(qt_cur, wq_sb, q_sb, 0, 0, hf)
                for hf in range(2):
                    qk_burst(kt_cur, wk_sb, kv_sb, 0, 0, hf)
                for k in range(4):
                    v_proj(1, k)

                # ---- main loop: pair outer, batch inner
                iters = [(p, b) for p in range(NPC) for b in range(BL)]
                qt_next = kt_next = None
                for it, (p, b) in enumerate(iters):
                    o_ps0 = psO.tile([HD + 1, L], f32, tag="o0", name=f"o0_{p}{b}")
                    o_ps1 = psO.tile([HD + 1, L], f32, tag="o1", name=f"o1_{p}{b}")

                    # prefetch next pair's ep tiles (slots free as this
                    # pair's b1 pass consumes the old ones)
                    if b == 1 and p + 1 < NPC:
                        for kc in range(KC):
                            for qh in range(2):
                                ep_dma(p + 1, kc, qh)

                    # reciprocals for the previous pair become available now
                    if b == 0 and p > 0:
                        r0 = (p - 1) * 4
                        nc.vector.reciprocal_approx_fast(
                            recip_t[p - 1][:], rs_t[p - 1][:]
                        )
                        nc.gpsimd.dma_start(rscr[r0:r0 + 4, :], recip_t[p - 1][:])

                    # filler jobs for this iteration, drained at k-ends
                    jobs = []
                    if (p, b) == (0, 0):
                        for k in range(4, KC):
                            jobs.append(lambda k=k: v_proj(1, k))
                    if it + 1 < len(iters):
                        np_, nb = iters[it + 1]
                        qt_next = qtp.tile([128, L], f16, tag="qt", name=f"qt{np_}{nb}")
                        kt_next = ktp.tile([128, L], f16, tag="kt", name=f"kt{np_}{nb}")
                        for hf in range(2):
                            jobs.append(
                                lambda hf=hf, t=qt_next, nb=nb, np_=np_:
                                qk_burst(t, wq_sb, q_sb, nb, np_, hf)
                            )
                        for hf in range(2):
                            jobs.append(
                                lambda hf=hf, t=kt_next, nb=nb, np_=np_:
                                qk_burst(t, wk_sb, kv_sb, nb, np_, hf)
                            )
                    if b == 0 and p > 0:
                        for b_ in range(BL):
                            for sub in range(2):
                                jobs.append(
                                    lambda b_=b_, sub=sub: normalize(p - 1, b_, sub)
                                )
                    if p == NPC - 1 and b == 1:
                        # (2,b0) normalize split in phases; b0 out-proj late so
                        # its PE work fills the tail-chain latency
                        jobs.append(lambda: normalize2_ph1(0, 0))
                        jobs.append(lambda: normalize2_ph1(0, 1))
                        jobs.append(lambda: normalize2_ph2(0, 0))
                        jobs.append(lambda: normalize2_ph2(0, 1))
                        for qc in range(KC):
                            jobs.append(lambda qc=qc: out_proj(0, qc))

                    nj = 0

                    def drain(n):
                        nonlocal nj
                        for _ in range(n):
                            if nj >= len(jobs):
                                return
                            jobs[nj]()
                            nj += 1

                    if p == NPC - 1 and b == 1:
                        ksched = [1, 1, 0, 0, 0, 1, 1, 3]
                    else:
                        ksched = [1 if len(jobs) <= KC else 2] * KC

                    h0, h1 = 2 * p, 2 * p + 1

                    # PV runs 2 units behind scores so the in-order PE queue
                    # never stalls on the exp->mul chain
                    pv_pend = []

                    def pv_issue(pk, pqh, ppt):
                        pqs = slice(pqh * 512, (pqh + 1) * 512)
                        nc.tensor.matmul(
                            o_ps0[:, pqs], Vt[b][pk][:, h0, :], ppt[:, 0:512],
                            start=(pk == 0), stop=(pk == KC - 1),
                        )
                        nc.tensor.matmul(
                            o_ps1[:, pqs], Vt[b][pk][:, h1, :], ppt[:, 512:L],
                            start=(pk == 0), stop=(pk == KC - 1),
                        )

                    for k in range(KC):
                        kt_sl = slice(k * 128, (k + 1) * 128)
                        for qh in range(2):
                            qs = slice(qh * 512, (qh + 1) * 512)
                            s = psS.tile([128, L], f32, tag="s", name=f"s{p}{b}{k}{qh}")
                            nc.tensor.matmul(
                                s[:, 0:512], kt_cur[0:64, kt_sl], qt_cur[0:64, qs]
                            )
                            nc.tensor.matmul(
                                s[:, 512:L], kt_cur[64:128, kt_sl], qt_cur[64:128, qs]
                            )
                            pr = prp.tile([128, L], bf16, tag="pr")
                            nc.scalar.activation(pr[:], s[:], AF.Exp, bias=expb[:])
                            pt = ptp.tile([128, L], bf16, tag="pt")
                            nc.vector.tensor_mul(pt[:], pr[:], ep_tiles[(p, k, qh)][:])
                            pv_pend.append((k, qh, pt))
                            if len(pv_pend) > 2:
                                pv_issue(*pv_pend.pop(0))
                        drain(ksched[k])
                    for args in pv_pend:
                        pv_issue(*args)
                    drain(len(jobs))

                    # evacuate o psum: copy to SBUF, export rowsum row
                    last_it = it == len(iters) - 1
                    for sub, o_ps in ((0, o_ps0), (1, o_ps1)):
                        xtu = xtup.tile(
                            [HD + 1, L], f32, tag="xtu", name=f"xtu{p}{b}{sub}"
                        )
                        if last_it:
                            # ACT is idle here
                            nc.scalar.copy(xtu[:], o_ps[:])
                        else:
                            nc.vector.tensor_copy(xtu[:], o_ps[:])
                        if p < 2:
                            rt, rr = rs_t[p], b * 2 + sub
                            nc.gpsimd.dma_start(rt[rr:rr + 1, :], xtu[HD:HD + 1, :])
                        xtu_map[(p, b, sub)] = xtu

                    qt_cur, kt_cur = qt_next, kt_next

                # ---- tail: last two heads' normalize, then b1 out-proj
                normalize2_ph1(1, 0)
                normalize2_ph1(1, 1)
                normalize2_ph2(1, 0)
                normalize2_ph2(1, 1)
                for qc in range(KC):
                    out_proj(1, qc)

    nc.compile()
    return nc


def _get_nc():
    if "nc" not in _CACHE:
        _CACHE["nc"] = _build()
    return _CACHE["nc"]


def _host_prep(q, kv, attn_pos, Wq, Wkv, Wproj, bproj):
    import ml_dtypes

    q = np.asarray(q, dtype=np.float32)
    kv = np.asarray(kv, dtype=np.float32)
    attn_pos = np.asarray(attn_pos, dtype=np.float32)
    Wq = np.asarray(Wq, dtype=np.float32)
    Wkv = np.asarray(Wkv, dtype=np.float32)
    Wproj = np.asarray(Wproj, dtype=np.float32)

    wq16 = np.ascontiguousarray((Wq * SCALE).T).astype(np.float16)   # [c, d]
    wk16 = np.ascontiguousarray(Wkv[:DIM].T).astype(np.float16)      # [c, d]
    wv16 = np.ascontiguousarray(Wkv[DIM:].T).astype(np.float16)      # [c, d]
    wp16 = np.ascontiguousarray(Wproj.T).astype(np.float16)          # [d, e]
    E = np.exp(attn_pos[0]).transpose(0, 2, 1)                       # [h, k, q]

    qT = np.ascontiguousarray(q.transpose(0, 2, 1)).astype(np.float16)
    kvT = np.ascontiguousarray(kv.transpose(0, 2, 1)).astype(np.float16)

    ep_g = []
    for g in range(2):
        Eg = E[g * HC:(g + 1) * HC]
        ep_g.append(
            np.ascontiguousarray(
                Eg.reshape(NPC, 2, KC, 128, 2, 512)
                .transpose(0, 2, 4, 3, 1, 5)
                .reshape(NPC, KC, 2, 128, L)
            ).astype(ml_dtypes.bfloat16)
        )
    w_g = []
    for g in range(2):
        hs = slice(g * HDIM, (g + 1) * HDIM)
        w_g.append({
            "wq": np.ascontiguousarray(wq16[:, hs]),
            "wk": np.ascontiguousarray(wk16[:, hs]),
            "wv": np.ascontiguousarray(wv16[:, hs]),
            "wp": np.ascontiguousarray(wp16[hs, :]),
        })

    in_maps = []
    for c in range(NCORES):
        bg, g = c // 2, c % 2
        m = dict(w_g[g])
        m["ep"] = ep_g[g]
        m["qT"] = np.ascontiguousarray(qT[2 * bg:2 * bg + 2])
        m["kvT"] = np.ascontiguousarray(kvT[2 * bg:2 * bg + 2])
        in_maps.append(m)
    return in_maps


def kernel(q, kv, attn_pos, Wq, Wkv, Wproj, bproj):
    from concourse.bass_utils import run_bass_kernel_spmd

    bproj = np.asarray(bproj, dtype=np.float32)
    nc = _get_nc()
    in_maps = _host_prep(q, kv, attn_pos, Wq, Wkv, Wproj, bproj)
    res = run_bass_kernel_spmd(nc, in_maps, list(range(NCORES)))
    out = np.empty((B, L, DIM), np.float32)
    for b in range(B):
        bg = b // 2
        out[b] = (
            res.results[2 * bg]["out"][b % 2].astype(np.float32)
            + res.results[2 * bg + 1]["out"][b % 2].astype(np.float32)
            + bproj[None, :]
        )
    return out
